# revision 4
# baseline (speedup 1.0000x reference)
"""Llama4-style MoE experts kernel for Trainium2 (Bass/Tile), expert-parallel
across 8 NeuronCores.

Math per expert e:
    gate_up = x_e @ W1_e          # (64,2048)@(2048,8192) -> (64,8192)
    gate, up = split(gate_up)     # (64,4096) each
    out_e   = (up * silu(gate)) @ W2_e   # (64,4096)@(4096,2048) -> (64,2048)

Sharding: experts 2c, 2c+1 go to core c (no cross-core communication).
Each core streams its 2 experts' weights (96.5 MB/expert) through SBUF once;
the kernel is HBM-bandwidth bound (~193 MB/core, ~540 us at ~358 GB/s).

Matmuls run in float32r (replicated-fp32 PE mode: full fp32 precision at
1 column/cycle for moving dim >= 256, vs 4 cycles/column for plain fp32).
"""

import numpy as np

import concourse.bass as bass
import concourse.mybir as mybir
import concourse.tile as tile
from concourse import bacc
from concourse.bass import ds
from concourse.bass_utils import run_bass_kernel_spmd
from concourse.masks import make_identity

# Problem shapes (hardcoded per contract).
E, T, H, I = 16, 64, 2048, 4096
NCORES = 8
EPC = E // NCORES  # experts per core = 2
P = 128
NT = 512           # free-dim tile (1 PSUM bank of fp32; fp32 moving-op max)
KSUB1 = H // P     # 16 k-subtiles for matmul 1
KSUB2 = I // P     # 32 k-subtiles for matmul 2
NJ = I // NT       # 8 gate/up column tiles
N2T = H // NT      # 4 output column tiles

F32 = mybir.dt.float32
F32R = mybir.dt.float32r


def build_program() -> bass.Bass:
    nc = bacc.Bacc(None, target_bir_lowering=False, debug=False)

    hidden = nc.dram_tensor("hidden_states", [EPC, T, H], F32, kind="ExternalInput")
    w1 = nc.dram_tensor("gate_up_proj", [EPC, H, 2 * I], F32, kind="ExternalInput")
    w2 = nc.dram_tensor("down_proj", [EPC, I, H], F32, kind="ExternalInput")
    out = nc.dram_tensor("out", [EPC, T, H], F32, kind="ExternalOutput")

    with tile.TileContext(nc) as tc:
        with (
            tc.tile_pool(name="const", bufs=1) as const,
            tc.tile_pool(name="wpool", bufs=3) as wpool,
            tc.tile_pool(name="xpool", bufs=2) as xpool,
            tc.tile_pool(name="xtpool", bufs=2) as xtpool,
            tc.tile_pool(name="htpool", bufs=2) as htpool,
            tc.tile_pool(name="spool", bufs=3) as spool,
            tc.tile_pool(name="opool", bufs=3) as opool,
            tc.tile_pool(name="mmps", bufs=4, space="PSUM") as mmps,
            tc.tile_pool(name="trps", bufs=2, space="PSUM") as trps,
        ):
            ident = const.tile([T, T], F32, name="ident")
            make_identity(nc, ident)

            for e in range(EPC):
                # ---- load x_e and transpose to [H-part, T] ----
                x_sb = xpool.tile([T, H], F32, name="x_sb", tag="x")
                nc.sync.dma_start(x_sb[:], hidden[e])

                xT = xtpool.tile([P, KSUB1, T], F32R, name="xT", tag="xT")
                for ko in range(KSUB1):
                    tp = trps.tile([P, T], F32, name="tp", tag="tp")
                    nc.tensor.transpose(tp[:], x_sb[:, ds(ko * P, P)], ident[:])
                    nc.vector.tensor_copy(xT[:, ko, :], tp[:])

                w1r = w1[e].rearrange("(ko p) n -> p ko n", p=P)
                w2r = w2[e].rearrange("(ko p) n -> p ko n", p=P)

                hT = htpool.tile([P, KSUB2, T], F32R, name="hT", tag="hT")

                # ---- matmul 1 + SwiGLU, one 512-wide column pair at a time ----
                for j in range(NJ):
                    wg = wpool.tile([P, KSUB1, NT], F32R, name="wg", tag="w")
                    nc.gpsimd.dma_start(wg[:], w1r[:, :, ds(j * NT, NT)])
                    wu = wpool.tile([P, KSUB1, NT], F32R, name="wu", tag="w")
                    nc.gpsimd.dma_start(wu[:], w1r[:, :, ds(I + j * NT, NT)])

                    gps = mmps.tile([T, NT], F32, name="gps", tag="mm")
                    for ko in range(KSUB1):
                        nc.tensor.matmul(
                            gps[:],
                            xT[:, ko, :],
                            wg[:, ko, :],
                            start=(ko == 0),
                            stop=(ko == KSUB1 - 1),
                        )
                    ups = mmps.tile([T, NT], F32, name="ups", tag="mm")
                    for ko in range(KSUB1):
                        nc.tensor.matmul(
                            ups[:],
                            xT[:, ko, :],
                            wu[:, ko, :],
                            start=(ko == 0),
                            stop=(ko == KSUB1 - 1),
                        )

                    sil = spool.tile([T, NT], F32, name="sil", tag="sil")
                    nc.scalar.activation(
                        sil[:], gps[:], mybir.ActivationFunctionType.Silu
                    )
                    h_sb = spool.tile([T, NT], F32, name="h_sb", tag="h")
                    nc.vector.tensor_mul(h_sb[:], sil[:], ups[:])

                    for i in range(NT // P):
                        tp2 = trps.tile([P, T], F32, name="tp2", tag="tp")
                        nc.tensor.transpose(
                            tp2[:], h_sb[:, ds(i * P, P)], ident[:]
                        )
                        nc.vector.tensor_copy(hT[:, (NT // P) * j + i, :], tp2[:])

                # ---- matmul 2: out_e = h @ W2_e ----
                for n2 in range(N2T):
                    wa = wpool.tile([P, KSUB1, NT], F32R, name="wa", tag="w")
                    nc.gpsimd.dma_start(wa[:], w2r[:, 0:KSUB1, ds(n2 * NT, NT)])
                    wb = wpool.tile([P, KSUB1, NT], F32R, name="wb", tag="w")
                    nc.gpsimd.dma_start(wb[:], w2r[:, KSUB1:KSUB2, ds(n2 * NT, NT)])

                    ops = mmps.tile([T, NT], F32, name="ops", tag="mm")
                    for ko in range(KSUB2):
                        wsrc = wa if ko < KSUB1 else wb
                        nc.tensor.matmul(
                            ops[:],
                            hT[:, ko, :],
                            wsrc[:, ko % KSUB1, :],
                            start=(ko == 0),
                            stop=(ko == KSUB2 - 1),
                        )
                    o_sb = opool.tile([T, NT], F32, name="o_sb", tag="o")
                    nc.scalar.copy(o_sb[:], ops[:])
                    nc.sync.dma_start(out[e][:, ds(n2 * NT, NT)], o_sb[:])

    nc.compile()
    return nc


_NC_CACHE = None


def _get_program():
    global _NC_CACHE
    if _NC_CACHE is None:
        _NC_CACHE = build_program()
    return _NC_CACHE


def run(inputs: dict, trace: bool = False):
    """Shard, run on 8 cores, gather. Returns (output, BassKernelResults)."""
    hs = np.ascontiguousarray(np.asarray(inputs["hidden_states"], dtype=np.float32))
    w1 = np.ascontiguousarray(np.asarray(inputs["gate_up_proj"], dtype=np.float32))
    w2 = np.ascontiguousarray(np.asarray(inputs["down_proj"], dtype=np.float32))

    in_maps = []
    for c in range(NCORES):
        sl = slice(c * EPC, (c + 1) * EPC)
        in_maps.append(
            {
                "hidden_states": hs[sl],
                "gate_up_proj": w1[sl],
                "down_proj": w2[sl],
            }
        )

    nc = _get_program()
    res = run_bass_kernel_spmd(nc, in_maps, core_ids=list(range(NCORES)), trace=trace)
    out = np.concatenate([r["out"] for r in res.results], axis=0)
    return out.astype(np.float32), res


def kernel(**inputs) -> np.ndarray:
    out, _ = run(inputs, trace=False)
    return out


# revision 8
# speedup vs baseline: 65.1293x; 65.1293x over previous
"""Llama4-style MoE experts kernel for Trainium2 (Bass/Tile), expert-parallel
across 8 NeuronCores.

Math per expert e:
    gate_up = x_e @ W1_e          # (64,2048)@(2048,8192) -> (64,8192)
    gate, up = split(gate_up)     # (64,4096) each
    out_e   = (up * silu(gate)) @ W2_e   # (64,4096)@(4096,2048) -> (64,2048)

Sharding: experts 2c, 2c+1 go to core c (no cross-core communication).
Each core streams its 2 experts' weights (96.5 MB/expert) through SBUF once;
the kernel is HBM-bandwidth bound (~194 MB/core, ~540 us at ~360 GB/s).

Weights are declared float32r in DRAM (bit-identical to fp32): the PE runs
replicated-fp32 matmuls at 1 column/cycle (vs 4 for plain fp32), and the
weight DMAs stay on the fast HWDGE ring (no casting). Weight DMAs move 8 MB
per dma_start with 4 KiB contiguous descriptors to saturate the DMA bus.
"""

import contextlib

import numpy as np

import concourse.bass as bass
import concourse.mybir as mybir
import concourse.tile as tile
from concourse import bacc
from concourse.bass import ds
from concourse.bass_utils import run_bass_kernel_spmd
from concourse.masks import make_identity

# Problem shapes (hardcoded per contract).
E, T, H, I = 16, 64, 2048, 4096
NCORES = 8
EPC = E // NCORES  # experts per core = 2
P = 128
NT = 512           # matmul free-dim tile (1 PSUM bank of fp32; fp32 max)
WT = 1024          # weight-DMA column tile (4 KiB contiguous per descriptor)
KSUB1 = H // P     # 16 k-subtiles for matmul 1
KSUB2 = I // P     # 32 k-subtiles for matmul 2
NJW = I // WT      # 4 gate/up DMA column tiles
N2W = H // WT      # 2 output DMA column tiles

F32 = mybir.dt.float32
F32R = mybir.dt.float32r


def build_program(repeat: int = 1) -> bass.Bass:
    """Build the per-core program. repeat>1 wraps the whole computation in a
    hardware loop (benchmarking only: amortizes PJRT dispatch overhead)."""
    nc = bacc.Bacc(None, target_bir_lowering=False, debug=False)

    hidden = nc.dram_tensor("hidden_states", [EPC, T, H], F32, kind="ExternalInput")
    w1 = nc.dram_tensor("gate_up_proj", [EPC, H, 2 * I], F32R, kind="ExternalInput")
    w2 = nc.dram_tensor("down_proj", [EPC, I, H], F32R, kind="ExternalInput")
    out = nc.dram_tensor("out", [EPC, T, H], F32, kind="ExternalOutput")

    with tile.TileContext(nc) as tc:
        with (
            tc.tile_pool(name="const", bufs=1) as const,
            tc.tile_pool(name="wpool", bufs=4) as wpool,
            tc.tile_pool(name="xpool", bufs=2) as xpool,
            tc.tile_pool(name="xtpool", bufs=2) as xtpool,
            tc.tile_pool(name="htpool", bufs=2) as htpool,
            tc.tile_pool(name="spool", bufs=3) as spool,
            tc.tile_pool(name="opool", bufs=3) as opool,
            tc.tile_pool(name="mmps", bufs=6, space="PSUM") as mmps,
            tc.tile_pool(name="trps", bufs=2, space="PSUM") as trps,
        ):
            ident = const.tile([T, T], F32, name="ident")
            make_identity(nc, ident)

            loop_cm = (
                tc.For_i(0, repeat, 1) if repeat > 1 else contextlib.nullcontext()
            )
            with loop_cm:
                body(nc, hidden, w1, w2, out, wpool, xpool, xtpool, htpool,
                     spool, opool, mmps, trps, ident)

    nc.compile()
    return nc


def body(nc, hidden, w1, w2, out, wpool, xpool, xtpool, htpool, spool,
         opool, mmps, trps, ident):
    KC = 8  # k-subtiles per weight DMA tile
    for e in range(EPC):
        # ---- load x_e and transpose to [H-part, T] ----
        x_sb = xpool.tile([T, H], F32, name="x_sb", tag="x")
        nc.sync.dma_start(x_sb[:], hidden[e])

        xT = xtpool.tile([P, KSUB1, T], F32R, name="xT", tag="xT")
        for ko in range(KSUB1):
            tp = trps.tile([P, T], F32, name="tp", tag="tp")
            nc.tensor.transpose(tp[:], x_sb[:, ds(ko * P, P)], ident[:])
            nc.vector.tensor_copy(xT[:, ko, :], tp[:])

        w1r = w1[e].rearrange("(ko p) n -> p ko n", p=P)
        w2r = w2[e].rearrange("(ko p) n -> p ko n", p=P)

        hT = htpool.tile([P, KSUB2, T], F32R, name="hT", tag="hT")

        # ---- matmul 1 + SwiGLU over 1024-wide column groups ----
        for j in range(NJW):
            ps = {}
            for src_i in range(2):          # 0 = gate, 1 = up
                for half in range(WT // NT):
                    nm = f"ps{src_i}{half}"
                    ps[src_i, half] = mmps.tile([T, NT], F32, name=nm, tag="mm")
            for src_i in range(2):
                base = src_i * I + j * WT
                for kc in range(KSUB1 // KC):
                    wt = wpool.tile([P, KC, WT], F32R, name="wt", tag="w")
                    nc.sync.dma_start(
                        wt[:], w1r[:, ds(kc * KC, KC), ds(base, WT)]
                    )
                    for half in range(WT // NT):
                        for k in range(KC):
                            ko = kc * KC + k
                            nc.tensor.matmul(
                                ps[src_i, half][:],
                                xT[:, ko, :],
                                wt[:, k, ds(half * NT, NT)],
                                start=(ko == 0),
                                stop=(ko == KSUB1 - 1),
                            )
            for half in range(WT // NT):
                sil = spool.tile([T, NT], F32, name="sil", tag="sil")
                nc.scalar.activation(
                    sil[:], ps[0, half][:], mybir.ActivationFunctionType.Silu
                )
                h_sb = spool.tile([T, NT], F32, name="h_sb", tag="h")
                nc.vector.tensor_mul(h_sb[:], sil[:], ps[1, half][:])

                for i in range(NT // P):
                    tp2 = trps.tile([P, T], F32, name="tp2", tag="tp")
                    nc.tensor.transpose(tp2[:], h_sb[:, ds(i * P, P)], ident[:])
                    kidx = (WT // P) * j + (NT // P) * half + i
                    nc.vector.tensor_copy(hT[:, kidx, :], tp2[:])

        # ---- matmul 2: out_e = h @ W2_e ----
        for n2 in range(N2W):
            ops = [
                mmps.tile([T, NT], F32, name=f"ops{h}", tag="mm")
                for h in range(WT // NT)
            ]
            for kc in range(KSUB2 // KC):
                wt2 = wpool.tile([P, KC, WT], F32R, name="wt2", tag="w")
                nc.sync.dma_start(
                    wt2[:], w2r[:, ds(kc * KC, KC), ds(n2 * WT, WT)]
                )
                for half in range(WT // NT):
                    for k in range(KC):
                        ko = kc * KC + k
                        nc.tensor.matmul(
                            ops[half][:],
                            hT[:, ko, :],
                            wt2[:, k, ds(half * NT, NT)],
                            start=(ko == 0),
                            stop=(ko == KSUB2 - 1),
                        )
            for half in range(WT // NT):
                o_sb = opool.tile([T, NT], F32, name="o_sb", tag="o")
                nc.scalar.copy(o_sb[:], ops[half][:])
                nc.sync.dma_start(
                    out[e][:, ds(n2 * WT + half * NT, NT)], o_sb[:]
                )


_NC_CACHE = None


def _get_program():
    global _NC_CACHE
    if _NC_CACHE is None:
        _NC_CACHE = build_program()
    return _NC_CACHE


def run(inputs: dict, trace: bool = False):
    """Shard, run on 8 cores, gather. Returns (output, BassKernelResults)."""
    hs = np.ascontiguousarray(np.asarray(inputs["hidden_states"], dtype=np.float32))
    w1 = np.ascontiguousarray(np.asarray(inputs["gate_up_proj"], dtype=np.float32))
    w2 = np.ascontiguousarray(np.asarray(inputs["down_proj"], dtype=np.float32))

    in_maps = []
    for c in range(NCORES):
        sl = slice(c * EPC, (c + 1) * EPC)
        in_maps.append(
            {
                "hidden_states": hs[sl],
                "gate_up_proj": w1[sl],
                "down_proj": w2[sl],
            }
        )

    nc = _get_program()
    res = run_bass_kernel_spmd(nc, in_maps, core_ids=list(range(NCORES)), trace=trace)
    out = np.concatenate([r["out"] for r in res.results], axis=0)
    return out.astype(np.float32), res


def kernel(**inputs) -> np.ndarray:
    out, _ = run(inputs, trace=False)
    return out


# revision 9
# speedup vs baseline: 129.6981x; 1.9914x over previous
"""Llama4-style MoE experts kernel for Trainium2 (Bass/Tile), expert-parallel
across 8 NeuronCores.

Math per expert e:
    gate_up = x_e @ W1_e          # (64,2048)@(2048,8192) -> (64,8192)
    gate, up = split(gate_up)     # (64,4096) each
    out_e   = (up * silu(gate)) @ W2_e   # (64,4096)@(4096,2048) -> (64,2048)

Sharding: experts 2c, 2c+1 go to core c (no cross-core communication).
Each core streams its 2 experts' weights (96.5 MB/expert) through SBUF once;
the kernel is HBM-bandwidth bound (~194 MB/core, ~540 us at ~360 GB/s).

Weights are declared float32r in DRAM (bit-identical to fp32): the PE runs
replicated-fp32 matmuls at 1 column/cycle (vs 4 for plain fp32), and the
weight DMAs stay on the fast HWDGE ring (no casting). Weight DMAs move 8 MB
per dma_start with 4 KiB contiguous descriptors to saturate the DMA bus.
"""

import contextlib

import numpy as np

import concourse.bass as bass
import concourse.mybir as mybir
import concourse.tile as tile
from concourse import bacc
from concourse.bass import ds
from concourse.bass_utils import run_bass_kernel_spmd
from concourse.masks import make_identity

# Problem shapes (hardcoded per contract).
E, T, H, I = 16, 64, 2048, 4096
NCORES = 8
EPC = E // NCORES  # experts per core = 2
P = 128
NT = 512           # matmul free-dim tile (1 PSUM bank of fp32; fp32 max)
WT = 1024          # weight-DMA column tile (4 KiB contiguous per descriptor)
KSUB1 = H // P     # 16 k-subtiles for matmul 1
KSUB2 = I // P     # 32 k-subtiles for matmul 2
NJW = I // WT      # 4 gate/up DMA column tiles
N2W = H // WT      # 2 output DMA column tiles

F32 = mybir.dt.float32
F32R = mybir.dt.float32r


def build_program(repeat: int = 1) -> bass.Bass:
    """Build the per-core program. repeat>1 wraps the whole computation in a
    hardware loop (benchmarking only: amortizes PJRT dispatch overhead)."""
    nc = bacc.Bacc(None, target_bir_lowering=False, debug=False)

    hidden = nc.dram_tensor("hidden_states", [EPC, T, H], F32, kind="ExternalInput")
    w1 = nc.dram_tensor("gate_up_proj", [EPC, H, 2 * I], F32R, kind="ExternalInput")
    w2 = nc.dram_tensor("down_proj", [EPC, I, H], F32R, kind="ExternalInput")
    out = nc.dram_tensor("out", [EPC, T, H], F32, kind="ExternalOutput")

    with tile.TileContext(nc) as tc:
        with (
            tc.tile_pool(name="const", bufs=1) as const,
            tc.tile_pool(name="wpool", bufs=4) as wpool,
            tc.tile_pool(name="xpool", bufs=2) as xpool,
            tc.tile_pool(name="xtpool", bufs=2) as xtpool,
            tc.tile_pool(name="htpool", bufs=2) as htpool,
            tc.tile_pool(name="spool", bufs=3) as spool,
            tc.tile_pool(name="opool", bufs=3) as opool,
            tc.tile_pool(name="mmps", bufs=6, space="PSUM") as mmps,
            tc.tile_pool(name="trps", bufs=2, space="PSUM") as trps,
        ):
            ident = const.tile([T, T], F32, name="ident")
            make_identity(nc, ident)

            loop_cm = (
                tc.For_i(0, repeat, 1) if repeat > 1 else contextlib.nullcontext()
            )
            with loop_cm:
                body(nc, hidden, w1, w2, out, wpool, xpool, xtpool, htpool,
                     spool, opool, mmps, trps, ident)

    nc.compile()
    return nc


def body(nc, hidden, w1, w2, out, wpool, xpool, xtpool, htpool, spool,
         opool, mmps, trps, ident):
    KC = 8  # k-subtiles per weight DMA tile
    for e in range(EPC):
        # ---- load x_e and transpose to [H-part, T] ----
        x_sb = xpool.tile([T, H], F32, name="x_sb", tag="x")
        nc.scalar.dma_start(x_sb[:], hidden[e])

        xT = xtpool.tile([P, KSUB1, T], F32R, name="xT", tag="xT")
        for ko in range(KSUB1):
            tp = trps.tile([P, T], F32, name="tp", tag="tp")
            nc.tensor.transpose(tp[:], x_sb[:, ds(ko * P, P)], ident[:])
            nc.vector.tensor_copy(xT[:, ko, :], tp[:])

        w1r = w1[e].rearrange("(ko p) n -> p ko n", p=P)
        w2r = w2[e].rearrange("(ko p) n -> p ko n", p=P)

        hT = htpool.tile([P, KSUB2, T], F32R, name="hT", tag="hT")

        # ---- matmul 1 + SwiGLU over 1024-wide column groups ----
        for j in range(NJW):
            ps = {}
            for src_i in range(2):          # 0 = gate, 1 = up
                for half in range(WT // NT):
                    nm = f"ps{src_i}{half}"
                    ps[src_i, half] = mmps.tile([T, NT], F32, name=nm, tag="mm")
            for src_i in range(2):
                base = src_i * I + j * WT
                for kc in range(KSUB1 // KC):
                    wt = wpool.tile([P, KC, WT], F32R, name="wt", tag="w")
                    nc.sync.dma_start(
                        wt[:], w1r[:, ds(kc * KC, KC), ds(base, WT)]
                    )
                    for half in range(WT // NT):
                        for k in range(KC):
                            ko = kc * KC + k
                            nc.tensor.matmul(
                                ps[src_i, half][:],
                                xT[:, ko, :],
                                wt[:, k, ds(half * NT, NT)],
                                start=(ko == 0),
                                stop=(ko == KSUB1 - 1),
                            )
            for half in range(WT // NT):
                sil = spool.tile([T, NT], F32, name="sil", tag="sil")
                nc.scalar.activation(
                    sil[:], ps[0, half][:], mybir.ActivationFunctionType.Silu
                )
                h_sb = spool.tile([T, NT], F32, name="h_sb", tag="h")
                nc.vector.tensor_mul(h_sb[:], sil[:], ps[1, half][:])

                for i in range(NT // P):
                    tp2 = trps.tile([P, T], F32, name="tp2", tag="tp")
                    nc.tensor.transpose(tp2[:], h_sb[:, ds(i * P, P)], ident[:])
                    kidx = (WT // P) * j + (NT // P) * half + i
                    nc.vector.tensor_copy(hT[:, kidx, :], tp2[:])

        # ---- matmul 2: out_e = h @ W2_e ----
        for n2 in range(N2W):
            ops = [
                mmps.tile([T, NT], F32, name=f"ops{h}", tag="mm")
                for h in range(WT // NT)
            ]
            for kc in range(KSUB2 // KC):
                wt2 = wpool.tile([P, KC, WT], F32R, name="wt2", tag="w")
                nc.sync.dma_start(
                    wt2[:], w2r[:, ds(kc * KC, KC), ds(n2 * WT, WT)]
                )
                for half in range(WT // NT):
                    for k in range(KC):
                        ko = kc * KC + k
                        nc.tensor.matmul(
                            ops[half][:],
                            hT[:, ko, :],
                            wt2[:, k, ds(half * NT, NT)],
                            start=(ko == 0),
                            stop=(ko == KSUB2 - 1),
                        )
            for half in range(WT // NT):
                o_sb = opool.tile([T, NT], F32, name="o_sb", tag="o")
                nc.scalar.copy(o_sb[:], ops[half][:])
                nc.scalar.dma_start(
                    out[e][:, ds(n2 * WT + half * NT, NT)], o_sb[:]
                )


_NC_CACHE = None


def _get_program():
    global _NC_CACHE
    if _NC_CACHE is None:
        _NC_CACHE = build_program()
    return _NC_CACHE


def run(inputs: dict, trace: bool = False):
    """Shard, run on 8 cores, gather. Returns (output, BassKernelResults)."""
    hs = np.ascontiguousarray(np.asarray(inputs["hidden_states"], dtype=np.float32))
    w1 = np.ascontiguousarray(np.asarray(inputs["gate_up_proj"], dtype=np.float32))
    w2 = np.ascontiguousarray(np.asarray(inputs["down_proj"], dtype=np.float32))

    in_maps = []
    for c in range(NCORES):
        sl = slice(c * EPC, (c + 1) * EPC)
        in_maps.append(
            {
                "hidden_states": hs[sl],
                "gate_up_proj": w1[sl],
                "down_proj": w2[sl],
            }
        )

    nc = _get_program()
    res = run_bass_kernel_spmd(nc, in_maps, core_ids=list(range(NCORES)), trace=trace)
    out = np.concatenate([r["out"] for r in res.results], axis=0)
    return out.astype(np.float32), res


def kernel(**inputs) -> np.ndarray:
    out, _ = run(inputs, trace=False)
    return out


# revision 10
# speedup vs baseline: 132.6932x; 1.0231x over previous
"""Llama4-style MoE experts kernel for Trainium2 (Bass/Tile), expert-parallel
across 8 NeuronCores.

Math per expert e:
    gate_up = x_e @ W1_e          # (64,2048)@(2048,8192) -> (64,8192)
    gate, up = split(gate_up)     # (64,4096) each
    out_e   = (up * silu(gate)) @ W2_e   # (64,4096)@(4096,2048) -> (64,2048)

Sharding: experts 2c, 2c+1 go to core c (no cross-core communication).
Each core streams its 2 experts' weights (96.5 MB/expert) through SBUF once;
the kernel is HBM-bandwidth bound (~194 MB/core, ~540 us at ~360 GB/s).

Weights are declared float32r in DRAM (bit-identical to fp32): the PE runs
replicated-fp32 matmuls at 1 column/cycle (vs 4 for plain fp32), and the
weight DMAs stay on the fast HWDGE ring (no casting). Weight DMAs move 8 MB
per dma_start with 4 KiB contiguous descriptors to saturate the DMA bus.
"""

import contextlib

import numpy as np

import concourse.bass as bass
import concourse.mybir as mybir
import concourse.tile as tile
from concourse import bacc
from concourse.bass import ds
from concourse.bass_utils import run_bass_kernel_spmd
from concourse.masks import make_identity

# Problem shapes (hardcoded per contract).
E, T, H, I = 16, 64, 2048, 4096
NCORES = 8
EPC = E // NCORES  # experts per core = 2
P = 128
NT = 512           # matmul free-dim tile (1 PSUM bank of fp32; fp32 max)
WT = 1024          # weight-DMA column tile (4 KiB contiguous per descriptor)
KSUB1 = H // P     # 16 k-subtiles for matmul 1
KSUB2 = I // P     # 32 k-subtiles for matmul 2
NJW = I // WT      # 4 gate/up DMA column tiles
N2W = H // WT      # 2 output DMA column tiles

F32 = mybir.dt.float32
F32R = mybir.dt.float32r


def build_program(repeat: int = 1) -> bass.Bass:
    """Build the per-core program. repeat>1 wraps the whole computation in a
    hardware loop (benchmarking only: amortizes PJRT dispatch overhead)."""
    nc = bacc.Bacc(None, target_bir_lowering=False, debug=False)

    hidden = nc.dram_tensor("hidden_states", [EPC, T, H], F32, kind="ExternalInput")
    w1 = nc.dram_tensor("gate_up_proj", [EPC, H, 2 * I], F32R, kind="ExternalInput")
    w2 = nc.dram_tensor("down_proj", [EPC, I, H], F32R, kind="ExternalInput")
    out = nc.dram_tensor("out", [EPC, T, H], F32, kind="ExternalOutput")

    with tile.TileContext(nc) as tc:
        with (
            tc.tile_pool(name="const", bufs=1) as const,
            tc.tile_pool(name="wpool", bufs=4) as wpool,
            tc.tile_pool(name="xpool", bufs=2) as xpool,
            tc.tile_pool(name="xtpool", bufs=2) as xtpool,
            tc.tile_pool(name="htpool", bufs=2) as htpool,
            tc.tile_pool(name="spool", bufs=3) as spool,
            tc.tile_pool(name="opool", bufs=3) as opool,
            tc.tile_pool(name="mmps", bufs=6, space="PSUM") as mmps,
            tc.tile_pool(name="trps", bufs=2, space="PSUM") as trps,
        ):
            ident = const.tile([T, T], F32, name="ident")
            make_identity(nc, ident)

            loop_cm = (
                tc.For_i(0, repeat, 1) if repeat > 1 else contextlib.nullcontext()
            )
            with loop_cm:
                body(nc, hidden, w1, w2, out, wpool, xpool, xtpool, htpool,
                     spool, opool, mmps, trps, ident)

    nc.compile()
    return nc


def body(nc, hidden, w1, w2, out, wpool, xpool, xtpool, htpool, spool,
         opool, mmps, trps, ident):
    KC = 8  # k-subtiles per weight DMA tile
    for e in range(EPC):
        # ---- load x_e and transpose to [H-part, T] ----
        x_sb = xpool.tile([T, H], F32, name="x_sb", tag="x")
        nc.gpsimd.dma_start(x_sb[:], hidden[e])

        xT = xtpool.tile([P, KSUB1, T], F32R, name="xT", tag="xT")
        for ko in range(KSUB1):
            tp = trps.tile([P, T], F32, name="tp", tag="tp")
            nc.tensor.transpose(tp[:], x_sb[:, ds(ko * P, P)], ident[:])
            nc.vector.tensor_copy(xT[:, ko, :], tp[:])

        w1r = w1[e].rearrange("(ko p) n -> p ko n", p=P)
        w2r = w2[e].rearrange("(ko p) n -> p ko n", p=P)

        hT = htpool.tile([P, KSUB2, T], F32R, name="hT", tag="hT")

        # ---- matmul 1 + SwiGLU over 1024-wide column groups ----
        for j in range(NJW):
            ps = {}
            for src_i in range(2):          # 0 = gate, 1 = up
                for half in range(WT // NT):
                    nm = f"ps{src_i}{half}"
                    ps[src_i, half] = mmps.tile([T, NT], F32, name=nm, tag="mm")
            for src_i in range(2):
                base = src_i * I + j * WT
                for kc in range(KSUB1 // KC):
                    wt = wpool.tile([P, KC, WT], F32R, name="wt", tag="w")
                    eng = nc.sync if (src_i * 2 + kc) % 2 == 0 else nc.scalar
                    eng.dma_start(
                        wt[:], w1r[:, ds(kc * KC, KC), ds(base, WT)]
                    )
                    for half in range(WT // NT):
                        for k in range(KC):
                            ko = kc * KC + k
                            nc.tensor.matmul(
                                ps[src_i, half][:],
                                xT[:, ko, :],
                                wt[:, k, ds(half * NT, NT)],
                                start=(ko == 0),
                                stop=(ko == KSUB1 - 1),
                            )
            for half in range(WT // NT):
                sil = spool.tile([T, NT], F32, name="sil", tag="sil")
                nc.scalar.activation(
                    sil[:], ps[0, half][:], mybir.ActivationFunctionType.Silu
                )
                h_sb = spool.tile([T, NT], F32, name="h_sb", tag="h")
                nc.vector.tensor_mul(h_sb[:], sil[:], ps[1, half][:])

                for i in range(NT // P):
                    tp2 = trps.tile([P, T], F32, name="tp2", tag="tp")
                    nc.tensor.transpose(tp2[:], h_sb[:, ds(i * P, P)], ident[:])
                    kidx = (WT // P) * j + (NT // P) * half + i
                    nc.vector.tensor_copy(hT[:, kidx, :], tp2[:])

        # ---- matmul 2: out_e = h @ W2_e ----
        for n2 in range(N2W):
            ops = [
                mmps.tile([T, NT], F32, name=f"ops{h}", tag="mm")
                for h in range(WT // NT)
            ]
            for kc in range(KSUB2 // KC):
                wt2 = wpool.tile([P, KC, WT], F32R, name="wt2", tag="w")
                eng = nc.sync if kc % 2 == 0 else nc.scalar
                eng.dma_start(
                    wt2[:], w2r[:, ds(kc * KC, KC), ds(n2 * WT, WT)]
                )
                for half in range(WT // NT):
                    for k in range(KC):
                        ko = kc * KC + k
                        nc.tensor.matmul(
                            ops[half][:],
                            hT[:, ko, :],
                            wt2[:, k, ds(half * NT, NT)],
                            start=(ko == 0),
                            stop=(ko == KSUB2 - 1),
                        )
            for half in range(WT // NT):
                o_sb = opool.tile([T, NT], F32, name="o_sb", tag="o")
                nc.scalar.copy(o_sb[:], ops[half][:])
                nc.gpsimd.dma_start(
                    out[e][:, ds(n2 * WT + half * NT, NT)], o_sb[:]
                )


_NC_CACHE = None


def _get_program():
    global _NC_CACHE
    if _NC_CACHE is None:
        _NC_CACHE = build_program()
    return _NC_CACHE


def run(inputs: dict, trace: bool = False):
    """Shard, run on 8 cores, gather. Returns (output, BassKernelResults)."""
    hs = np.ascontiguousarray(np.asarray(inputs["hidden_states"], dtype=np.float32))
    w1 = np.ascontiguousarray(np.asarray(inputs["gate_up_proj"], dtype=np.float32))
    w2 = np.ascontiguousarray(np.asarray(inputs["down_proj"], dtype=np.float32))

    in_maps = []
    for c in range(NCORES):
        sl = slice(c * EPC, (c + 1) * EPC)
        in_maps.append(
            {
                "hidden_states": hs[sl],
                "gate_up_proj": w1[sl],
                "down_proj": w2[sl],
            }
        )

    nc = _get_program()
    res = run_bass_kernel_spmd(nc, in_maps, core_ids=list(range(NCORES)), trace=trace)
    out = np.concatenate([r["out"] for r in res.results], axis=0)
    return out.astype(np.float32), res


def kernel(**inputs) -> np.ndarray:
    out, _ = run(inputs, trace=False)
    return out
